# revision 1
# baseline (speedup 1.0000x reference)
"""Trainium2 Bass kernel for NodeGraphTransformerLayer (GNN message passing).

Strategy (8 NeuronCores, SPMD single program):
  - Pad node space to NPAD = 8 * NPC (NPC = nwin*128). Core c owns nodes
    [c*NPC, (c+1)*NPC) and ALL edges whose dst falls in that range, sorted by
    dst. No cross-core reduction needed: each core computes its nodes' full
    output rows.
  - Host prep ("sharding"): partition + sort edges per core, pad each
    128-node window's edge list to a uniform number of 128-edge blocks,
    send each core its edges' spatial rows (transposed), src indices, and
    local dst ids. Padding edges carry dst = -1 so they drop out of the
    one-hot segment sums.
  - Device per core:
    Phase 1: KV table [NPAD, 512] = h @ [Wk|Wv] + b (replicated on all
      cores, feeds gathers); Q slice for own nodes, pre-scaled by
      1/sqrt(HD), kept resident in SBUF.
    Phase 2 (per window w, per 128-edge block): indirect-DMA gather
      KV[src]; one-hot(dstT) matmul expands Q[dst]; per-edge score
      s = sum_h K*Q + spatial@Wsp_r + bsp_r, clipped, exp'd on ScalarE;
      messages [V*score | score] reduced into a PSUM accumulator via
      one-hot(dst) matmuls (segment sum without scatter).
    Phase 3 (per window): h_attn = wV/(z+eps) (channel-major), sigmoid
      gate, Wo, residual+LN1+BN1, FFN (gelu), residual+LN2+BN2, DMA out.
"""

import math
import sys
from contextlib import ExitStack

import numpy as np

sys.path.insert(0, "/opt/trn_rl_repo")

import concourse.bass as bass
import concourse.tile as tile
from concourse import bacc, mybir
from concourse.bass import IndirectOffsetOnAxis
from concourse.bass_utils import run_bass_kernel_spmd

F32 = mybir.dt.float32
F16 = mybir.dt.float16
I32 = mybir.dt.int32
AF = mybir.ActivationFunctionType
ALU = mybir.AluOpType
AX = mybir.AxisListType

N, E, DIN, DOUT, H, HD, FF = 50000, 800000, 256, 256, 8, 32, 1024
NCORES = 8
SCALE = float(np.sqrt(DOUT // H))
EPS_LN = 1e-5
EPS_BN = 1e-5


class Cfg:
    def __init__(self, nwin, bmax, ncores=NCORES, npad=None, ln_fold=None):
        self.ncores = ncores
        self.nwin = nwin              # 128-node windows per core
        self.bmax = bmax              # 128-edge blocks per window (uniform)
        self.npc = nwin * 128         # padded nodes per core
        self.npad = npad if npad is not None else self.npc * ncores
        self.EPW = bmax * 128         # edge slots per window
        self.EP = nwin * self.EPW     # edge slots per core


def build(cfg: Cfg):
    nc = bacc.Bacc("TRN2", target_bir_lowering=False, debug=False,
                   num_devices=cfg.ncores)

    def inp(name, shape, dtype=F32):
        return nc.dram_tensor(name, list(shape), dtype, kind="ExternalInput")

    h_T = inp("h_T", [256, cfg.npad], F16)
    hsT = inp("hsT", [256, cfg.npc])
    h_sl = inp("h_sl", [cfg.npc, 256])          # h slice + bo
    spT_d = inp("spT", [256, cfg.EP], F16)
    dstseq = inp("dstseq", [1, cfg.EP])
    dstcol_d = inp("dstcol", [128, cfg.nwin * cfg.bmax])
    srci_d = inp("srci", [128, cfg.nwin * cfg.bmax], I32)
    Wkv = inp("Wkv", [256, 512], F16); bkv_row = inp("bkv_row", [1, 512])
    Wq = inp("Wq", [256, 256]); bq_row = inp("bq_row", [1, 256])
    Wsp = inp("Wsp", [256, 8], F16); bsp_row = inp("bsp_row", [1, 8])
    Wg = inp("Wg", [512, 256]); bgc = inp("bgc", [128, 2])
    Wo = inp("Wo", [256, 256])
    W1 = inp("W1", [256, 1024]); b1c = inp("b1c", [128, 8])
    W2 = inp("W2", [1024, 256]); b2 = inp("b2", [128, 256])
    cs1 = inp("cs1", [128, 256]); cb1 = inp("cb1", [128, 256])
    cs2 = inp("cs2", [128, 256]); cb2 = inp("cb2", [128, 256])
    iota_r = inp("iota_r", [128, 128]); iota_c = inp("iota_c", [128, 1])
    ident = inp("ident", [128, 128]); ehead = inp("ehead", [8, 256])
    ones_row = inp("ones_row", [1, 512])
    out_d = nc.dram_tensor("out", [cfg.npc, 256], F32, kind="ExternalOutput")
    kvt = nc.dram_tensor("kv_table", [cfg.npad, 512], F16)

    with tile.TileContext(nc) as tc, ExitStack() as ctx:
        const = ctx.enter_context(tc.tile_pool(name="const", bufs=1))

        def ctile(src, shape, dtype=F32, tag=None, rearr=None):
            t = const.tile(list(shape), dtype, tag=tag or src.name)
            s = src[:]
            if rearr is not None:
                s = s.rearrange(rearr[0], **rearr[1])
            nc.sync.dma_start(t[:], s)
            return t

        kvw = ctile(Wkv, [128, 2, 512], dtype=F16, rearr=("(s p) n -> p s n", dict(p=128)))
        qw = ctile(Wq, [128, 2, 256], rearr=("(s p) n -> p s n", dict(p=128)))
        spw = ctile(Wsp, [128, 2, 8], dtype=F16, rearr=("(s p) n -> p s n", dict(p=128)))
        wgw = ctile(Wg, [128, 4, 256], rearr=("(s p) n -> p s n", dict(p=128)))
        wow = ctile(Wo, [128, 2, 256], rearr=("(s p) n -> p s n", dict(p=128)))
        w1w = ctile(W1, [128, 2, 1024], rearr=("(s p) n -> p s n", dict(p=128)))
        w2w = ctile(W2, [128, 8, 256], rearr=("(s p) n -> p s n", dict(p=128)))
        bkvr = ctile(bkv_row, [1, 512])
        bqr = ctile(bq_row, [1, 256])
        bspr = ctile(bsp_row, [1, 8])
        bgct = ctile(bgc, [128, 2])
        b1ct = ctile(b1c, [128, 8])
        b2t = ctile(b2, [128, 256])
        cs1t = ctile(cs1, [128, 256]); cb1t = ctile(cb1, [128, 256])
        cs2t = ctile(cs2, [128, 256]); cb2t = ctile(cb2, [128, 256])
        iotar = ctile(iota_r, [128, 128]); iotac = ctile(iota_c, [128, 1])
        idt = ctile(ident, [128, 128]); eh = ctile(ehead, [8, 256])
        onesr = ctile(ones_row, [1, 512])
        srci_sb = ctile(srci_d, [128, cfg.nwin * cfg.bmax], I32)
        dstc_sb = ctile(dstcol_d, [128, cfg.nwin * cfg.bmax])
        qall = const.tile([128, cfg.nwin * 256], F16, tag="qall")
        zcol = const.tile([128, 1], F32, tag="zcol")
        nc.gpsimd.memset(zcol[:], 0.0)
        epscol = const.tile([128, 1], F32, tag="epscol")
        nc.gpsimd.memset(epscol[:], EPS_LN)
        nc.const_aps.aps[(F32, 0.0)] = zcol[:]
        nc.const_aps.aps[(F32, EPS_LN)] = epscol[:]

        # ---------------- phase 1: KV table + resident Q ----------------
        ST = 1024
        while cfg.npad % ST != 0:
            ST //= 2
        with tc.tile_pool(name="p1", bufs=2) as p1, \
             tc.tile_pool(name="p1ps", bufs=2, space="PSUM") as p1ps, \
             tc.tile_pool(name="p1o", bufs=3) as p1o:
            for s in range(cfg.npad // ST):
                ht = p1.tile([128, 2, ST], F16, tag="ht")
                nc.sync.dma_start(ht[:, 0, :], h_T[0:128, s * ST:(s + 1) * ST])
                nc.sync.dma_start(ht[:, 1, :], h_T[128:256, s * ST:(s + 1) * ST])
                for t in range(ST // 128):
                    ps = p1ps.tile([128, 512], F32, tag="kvps")
                    nc.tensor.matmul(ps[:], lhsT=ht[:, 0, t * 128:(t + 1) * 128],
                                     rhs=kvw[:, 0, :], start=True, stop=False)
                    nc.tensor.matmul(ps[:], lhsT=ht[:, 1, t * 128:(t + 1) * 128],
                                     rhs=kvw[:, 1, :], start=False, stop=False)
                    nc.tensor.matmul(ps[:], lhsT=onesr[0:1, 0:128],
                                     rhs=bkvr[0:1, :], start=False, stop=True)
                    ot = p1o.tile([128, 512], F16, tag="kvo")
                    nc.scalar.activation(out=ot[:], in_=ps[:], func=AF.Copy)
                    r0 = s * ST + t * 128
                    nc.sync.dma_start(kvt[r0:r0 + 128, :], ot[:])
            for w in range(cfg.nwin):
                hst = p1.tile([128, 2, 128], F32, tag="hst")
                nc.sync.dma_start(hst[:, 0, :], hsT[0:128, w * 128:(w + 1) * 128])
                nc.sync.dma_start(hst[:, 1, :], hsT[128:256, w * 128:(w + 1) * 128])
                ps = p1ps.tile([128, 256], F32, tag="qps")
                nc.tensor.matmul(ps[:], lhsT=hst[:, 0, :], rhs=qw[:, 0, :],
                                 start=True, stop=False)
                nc.tensor.matmul(ps[:], lhsT=hst[:, 1, :], rhs=qw[:, 1, :],
                                 start=False, stop=False)
                nc.tensor.matmul(ps[:], lhsT=onesr[0:1, 0:128],
                                 rhs=bqr[0:1, :], start=False, stop=True)
                nc.scalar.activation(out=qall[:, w * 256:(w + 1) * 256],
                                     in_=ps[:], func=AF.Copy)

        # ---------------- phase 2+3 ----------------
        p2 = ctx.enter_context(tc.tile_pool(name="p2", bufs=2))
        kvp = ctx.enter_context(tc.tile_pool(name="kvgp", bufs=8))
        ps_wv = ctx.enter_context(tc.tile_pool(name="ps_wv", bufs=1, space="PSUM"))
        ps_sp = ctx.enter_context(tc.tile_pool(name="ps_sp", bufs=2, space="PSUM"))
        ps_qe = ctx.enter_context(tc.tile_pool(name="ps_qe", bufs=2, space="PSUM"))
        ps_b = ctx.enter_context(tc.tile_pool(name="ps_b", bufs=2, space="PSUM"))
        ps_g1 = ctx.enter_context(tc.tile_pool(name="ps_g1", bufs=1, space="PSUM"))
        p3 = ctx.enter_context(tc.tile_pool(name="p3", bufs=2))
        p3b = ctx.enter_context(tc.tile_pool(name="p3b", bufs=2))

        def layernorm(xin, cst, cbt):
            mu = p3.tile([128, 1], F32, tag="mu")
            nc.vector.tensor_reduce(out=mu[:], in_=xin[:], axis=AX.X, op=ALU.add)
            nc.vector.tensor_scalar_mul(out=mu[:], in0=mu[:], scalar1=1.0 / 256)
            xc = p3.tile([128, 256], F32, tag="xc")
            nc.vector.tensor_scalar(out=xc[:], in0=xin[:], scalar1=mu[:, 0:1],
                                    scalar2=None, op0=ALU.subtract)
            sq = p3.tile([128, 256], F32, tag="sq")
            nc.vector.tensor_tensor(out=sq[:], in0=xc[:], in1=xc[:], op=ALU.mult)
            vs = p3.tile([128, 1], F32, tag="vs")
            nc.vector.tensor_reduce(out=vs[:], in_=sq[:], axis=AX.X, op=ALU.add)
            sd = p3.tile([128, 1], F32, tag="sd")
            nc.scalar.activation(out=sd[:], in_=vs[:], func=AF.Sqrt,
                                 scale=1.0 / 256, bias=EPS_LN)
            rstd = p3.tile([128, 1], F32, tag="rstd")
            nc.vector.reciprocal(out=rstd[:], in_=sd[:])
            xn = p3.tile([128, 256], F32, tag="xn")
            nc.vector.tensor_scalar(out=xn[:], in0=xc[:], scalar1=rstd[:, 0:1],
                                    scalar2=None, op0=ALU.mult)
            o = p3.tile([128, 256], F32, tag="lno")
            nc.vector.tensor_tensor(out=o[:], in0=xn[:], in1=cst[:], op=ALU.mult)
            nc.vector.tensor_tensor(out=o[:], in0=o[:], in1=cbt[:], op=ALU.add)
            return o

        for w in range(cfg.nwin):
            spt = p2.tile([128, 2, cfg.EPW], F16, tag="spt")
            nc.sync.dma_start(spt[:, 0, :], spT_d[0:128, w * cfg.EPW:(w + 1) * cfg.EPW])
            nc.sync.dma_start(spt[:, 1, :], spT_d[128:256, w * cfg.EPW:(w + 1) * cfg.EPW])
            dstb = p2.tile([128, cfg.EPW], F32, tag="dstb")
            nc.sync.dma_start(
                dstb[:], dstseq[0:1, w * cfg.EPW:(w + 1) * cfg.EPW].partition_broadcast(128))
            wv = ps_wv.tile([128, 384], F32, tag="wv")
            for g0 in range(0, cfg.bmax, 4):
                gs = min(4, cfg.bmax - g0)
                sp8g = ps_sp.tile([128, 32], F32, tag="sp8")
                for j in range(gs):
                    b = g0 + j
                    sl = sp8g[:, j * 8:(j + 1) * 8]
                    nc.tensor.matmul(sl, lhsT=spt[:, 0, b * 128:(b + 1) * 128],
                                     rhs=spw[:, 0, :], start=(j == 0), stop=False,
                                     skip_group_check=True)
                    nc.tensor.matmul(sl, lhsT=spt[:, 1, b * 128:(b + 1) * 128],
                                     rhs=spw[:, 1, :], start=False, stop=False,
                                     skip_group_check=True)
                    nc.tensor.matmul(sl, lhsT=onesr[0:1, 0:128],
                                     rhs=bspr[0:1, :], start=False,
                                     stop=(j == gs - 1), skip_group_check=True)
                ohT4 = p2.tile([128, 4, 128], F16, tag="ohT")
                nc.vector.tensor_tensor(
                    out=ohT4[:, 0:gs, :],
                    in0=iotac[:].to_broadcast([128, gs * 128]).rearrange(
                        "p (g n) -> p g n", n=128),
                    in1=dstb[:, g0 * 128:(g0 + gs) * 128].rearrange(
                        "p (g n) -> p g n", n=128),
                    op=ALU.is_equal)
                oh4 = p2.tile([128, 4, 128], F16, tag="oh")
                nc.vector.tensor_tensor(
                    out=oh4[:, 0:gs, :],
                    in0=dstc_sb[:, w * cfg.bmax + g0:w * cfg.bmax + g0 + gs]
                        .rearrange("p (g o) -> p g o", o=1)
                        .to_broadcast([128, gs, 128]),
                    in1=iotar[:].rearrange("p (o n) -> p o n", o=1)
                        .to_broadcast([128, gs, 128]),
                    op=ALU.is_equal)
                s84 = p2.tile([128, 4, 8], F32, tag="s84")
                mext4 = p2.tile([128, 4, 264], F16, tag="mext")
                kvgs = []
                for j in range(gs):
                    col = w * cfg.bmax + g0 + j
                    kvg = kvp.tile([128, 512], F16, tag="kvg")
                    nc.gpsimd.indirect_dma_start(
                        out=kvg[:], out_offset=None, in_=kvt[:],
                        in_offset=IndirectOffsetOnAxis(ap=srci_sb[:, col:col + 1], axis=0))
                    kvgs.append(kvg)
                    qe = ps_qe.tile([128, 256], F32, tag="qe")
                    nc.tensor.matmul(qe[:], lhsT=ohT4[:, j, :],
                                     rhs=qall[:, w * 256:(w + 1) * 256],
                                     start=True, stop=True)
                    tsb = p2.tile([128, 256], F32, tag="tsb")
                    nc.vector.tensor_tensor(out=tsb[:], in0=kvg[:, 0:256],
                                            in1=qe[:], op=ALU.mult)
                    nc.vector.tensor_reduce(
                        out=s84[:, j, :], in_=tsb[:].rearrange("p (h d) -> p h d", d=32),
                        axis=AX.X, op=ALU.add)
                sst4 = p2.tile([128, 4, 8], F32, tag="sst4")
                nc.vector.tensor_tensor(
                    out=sst4[:, 0:gs, :], in0=s84[:, 0:gs, :],
                    in1=sp8g[:].rearrange("p (g h) -> p g h", h=8)[:, 0:gs, :],
                    op=ALU.add)
                nc.vector.tensor_scalar(out=sst4[:, 0:gs, :], in0=sst4[:, 0:gs, :],
                                        scalar1=5.0, scalar2=-5.0,
                                        op0=ALU.min, op1=ALU.max)
                nc.scalar.activation(out=mext4[:, 0:gs, 256:264],
                                     in_=sst4[:, 0:gs, :], func=AF.Exp)
                for j in range(gs):
                    b = g0 + j
                    nc.vector.tensor_tensor(
                        out=mext4[:, j, 0:256].rearrange("p (h d) -> p h d", d=32),
                        in0=kvgs[j][:, 256:512].rearrange("p (h d) -> p h d", d=32),
                        in1=mext4[:, j, 256:264].to_broadcast([128, 8, 32]),
                        op=ALU.mult)
                    st = b == 0
                    fin = b == cfg.bmax - 1
                    nc.tensor.matmul(wv[:, 0:128], lhsT=mext4[:, j, 0:128],
                                     rhs=oh4[:, j, :], start=st, stop=False,
                                     skip_group_check=True)
                    nc.tensor.matmul(wv[:, 128:256], lhsT=mext4[:, j, 128:256],
                                     rhs=oh4[:, j, :], start=False, stop=False,
                                     skip_group_check=True)
                    nc.tensor.matmul(wv[0:8, 256:384], lhsT=mext4[:, j, 256:264],
                                     rhs=oh4[:, j, :], start=False, stop=fin,
                                     skip_group_check=True)

            # ---------------- phase 3 ----------------
            zr = p3.tile([8, 128], F32, tag="zr")
            nc.vector.tensor_scalar(out=zr[:], in0=wv[0:8, 256:384], scalar1=1e-6,
                                    scalar2=None, op0=ALU.add)
            zrr = p3.tile([8, 128], F32, tag="zrr")
            nc.vector.reciprocal(out=zrr[:], in_=zr[:])
            zrep = ps_b.tile([128, 256], F32, tag="psb")
            nc.tensor.matmul(zrep[:, 0:128], lhsT=eh[0:8, 0:128], rhs=zrr[:],
                             start=True, stop=False)
            nc.tensor.matmul(zrep[:, 128:256], lhsT=eh[0:8, 128:256], rhs=zrr[:],
                             start=False, stop=True)
            zrs = p3.tile([128, 256], F32, tag="zrs")
            nc.scalar.activation(out=zrs[:], in_=zrep[:], func=AF.Copy)
            hat = p3.tile([128, 256], F32, tag="hat")
            nc.vector.tensor_tensor(out=hat[:], in0=wv[:, 0:256], in1=zrs[:],
                                    op=ALU.mult)
            hstw = p3b.tile([128, 2, 128], F32, tag="hstw")
            nc.sync.dma_start(hstw[:, 0, :], hsT[0:128, w * 128:(w + 1) * 128])
            nc.sync.dma_start(hstw[:, 1, :], hsT[128:256, w * 128:(w + 1) * 128])
            gate = ps_b.tile([128, 256], F32, tag="psb")
            rhs_list = [hstw[:, 0, :], hstw[:, 1, :], hat[:, 0:128], hat[:, 128:256]]
            for ci, rr in enumerate(rhs_list):
                for co in range(2):
                    nc.tensor.matmul(gate[:, co * 128:(co + 1) * 128],
                                     lhsT=wgw[:, ci, co * 128:(co + 1) * 128], rhs=rr,
                                     start=(ci == 0 and co == 0), stop=(ci == 3 and co == 1),
                                     skip_group_check=True)
            gts = p3.tile([128, 256], F32, tag="gts")
            nc.scalar.activation(out=gts[:, 0:128], in_=gate[:, 0:128],
                                 func=AF.Sigmoid, bias=bgct[:, 0:1])
            nc.scalar.activation(out=gts[:, 128:256], in_=gate[:, 128:256],
                                 func=AF.Sigmoid, bias=bgct[:, 1:2])
            x1 = p3.tile([128, 256], F32, tag="x1")
            nc.vector.tensor_tensor(out=x1[:], in0=gts[:], in1=hat[:], op=ALU.mult)
            yps = ps_b.tile([128, 256], F32, tag="psb")
            nc.tensor.matmul(yps[:], lhsT=x1[:, 0:128], rhs=wow[:, 0, :],
                             start=True, stop=False)
            nc.tensor.matmul(yps[:], lhsT=x1[:, 128:256], rhs=wow[:, 1, :],
                             start=False, stop=True)
            hwin = p3b.tile([128, 256], F32, tag="hwin")
            nc.sync.dma_start(hwin[:], h_sl[w * 128:(w + 1) * 128, :])
            x = p3.tile([128, 256], F32, tag="x")
            nc.vector.tensor_tensor(out=x[:], in0=yps[:], in1=hwin[:], op=ALU.add)
            x2in = layernorm(x, cs1t, cb1t)
            xT = ps_b.tile([128, 256], F32, tag="psb")
            nc.tensor.matmul(xT[:, 0:128], lhsT=x2in[:, 0:128], rhs=idt[:],
                             is_transpose=True, start=True, stop=False)
            nc.tensor.matmul(xT[:, 128:256], lhsT=x2in[:, 128:256], rhs=idt[:],
                             is_transpose=True, start=False, stop=True)
            xTs = p3.tile([128, 256], F32, tag="xTs")
            nc.scalar.activation(out=xTs[:], in_=xT[:], func=AF.Copy)
            g1s = p3.tile([128, 1024], F32, tag="g1s")
            for half in range(2):
                g1 = ps_g1.tile([128, 512], F32, tag="psg1")
                for q in range(4):
                    ct = half * 4 + q
                    off = q * 128
                    nc.tensor.matmul(g1[:, off:off + 128],
                                     lhsT=w1w[:, 0, ct * 128:(ct + 1) * 128],
                                     rhs=xTs[:, 0:128], start=(q == 0), stop=False,
                                     skip_group_check=True)
                    nc.tensor.matmul(g1[:, off:off + 128],
                                     lhsT=w1w[:, 1, ct * 128:(ct + 1) * 128],
                                     rhs=xTs[:, 128:256], start=False,
                                     stop=(q == 3), skip_group_check=True)
                for q in range(4):
                    ct = half * 4 + q
                    nc.scalar.activation(out=g1s[:, ct * 128:(ct + 1) * 128],
                                         in_=g1[:, q * 128:(q + 1) * 128],
                                         func=AF.Gelu, bias=b1ct[:, ct:ct + 1])
            x2p = ps_b.tile([128, 256], F32, tag="psb")
            for ct in range(8):
                nc.tensor.matmul(x2p[:], lhsT=g1s[:, ct * 128:(ct + 1) * 128],
                                 rhs=w2w[:, ct, :], start=(ct == 0), stop=(ct == 7))
            x3 = p3.tile([128, 256], F32, tag="x3")
            nc.vector.tensor_tensor(out=x3[:], in0=x2p[:], in1=x2in[:], op=ALU.add)
            nc.vector.tensor_tensor(out=x3[:], in0=x3[:], in1=b2t[:], op=ALU.add)
            xo = layernorm(x3, cs2t, cb2t)
            nc.sync.dma_start(out_d[w * 128:(w + 1) * 128, :], xo[:])

    nc.compile()
    return nc


def prepare(cfg: Cfg, inputs, n_real, e_real):
    """Host-side sharding: returns in_maps (list of dicts per core)."""
    f32 = np.float32
    h = np.asarray(inputs["h"], f32)
    sp = np.asarray(inputs["spatial_pos"], f32)
    src = np.asarray(inputs["src"]).astype(np.int64)
    dst = np.asarray(inputs["dst"]).astype(np.int64)
    W = {k: np.asarray(inputs[k], f32) for k in
         ["Wq", "bq", "Wk", "bk", "Wv", "bv", "Wsp", "bsp", "Wo", "bo",
          "Wg", "bg", "W1", "b1", "W2", "b2", "ln1_g", "ln1_b", "ln2_g",
          "ln2_b", "bn1_g", "bn1_b", "bn2_g", "bn2_b"]}

    npc, npad = cfg.npc, cfg.npad
    h_pad = np.zeros((npad, 256), f32)
    h_pad[:n_real] = h

    Wkv = np.concatenate([W["Wk"], W["Wv"]], 1)
    bkv = np.concatenate([W["bk"], W["bv"]])
    Wq_s = W["Wq"] / SCALE
    bq_s = W["bq"] / SCALE
    Wsp_r = W["Wsp"].astype(np.float64).reshape(256, 8, 32).sum(-1).astype(f32)
    bsp_r = W["bsp"].astype(np.float64).reshape(8, 32).sum(-1).astype(f32)
    # reorder Wg rows: device concat layout [h(256) | h_attn(256)] ->
    # reference layout interleaved per head (h-head, attn-head)
    pr = np.empty(512, np.int64)
    r = np.arange(256)
    pr[:256] = (r // 32) * 64 + (r % 32)
    pr[256:] = (r // 32) * 64 + 32 + (r % 32)
    Wg_r = W["Wg"][pr]
    rs = 1.0 / np.sqrt(np.float32(1.0 + EPS_BN))
    cs1 = W["ln1_g"] * rs * W["bn1_g"]
    cb1 = W["ln1_b"] * rs * W["bn1_g"] + W["bn1_b"]
    cs2 = W["ln2_g"] * rs * W["bn2_g"]
    cb2 = W["ln2_b"] * rs * W["bn2_g"] + W["bn2_b"]

    rep = lambda v: np.tile(np.asarray(v, f32)[None, :], (128, 1))
    ehead = np.zeros((8, 256), f32)
    ehead[np.arange(256) // 32, np.arange(256)] = 1.0

    shared = dict(
        h_T=np.ascontiguousarray(h_pad.T).astype(np.float16),
        Wkv=Wkv.astype(np.float16), bkv_row=bkv[None, :].astype(f32),
        Wq=Wq_s, bq_row=bq_s[None, :].astype(f32),
        Wsp=Wsp_r.astype(np.float16), bsp_row=bsp_r[None, :],
        Wg=Wg_r, bgc=np.ascontiguousarray(W["bg"].reshape(2, 128).T),
        Wo=W["Wo"],
        W1=W["W1"], b1c=np.ascontiguousarray(W["b1"].reshape(8, 128).T),
        W2=W["W2"], b2=rep(W["b2"]),
        cs1=rep(cs1), cb1=rep(cb1), cs2=rep(cs2), cb2=rep(cb2),
        iota_r=np.tile(np.arange(128, dtype=f32), (128, 1)),
        iota_c=np.arange(128, dtype=f32)[:, None],
        ident=np.eye(128, dtype=f32),
        ehead=ehead,
        ones_row=np.ones((1, 512), f32),
    )

    core_of = dst // npc
    in_maps = []
    for c in range(cfg.ncores):
        em = np.nonzero(core_of == c)[0]
        dl = (dst[em] - c * npc).astype(np.int64)
        order = np.argsort(dl, kind="stable")
        em = em[order]
        dl = dl[order]
        wi = dl >> 7
        cnt = np.bincount(wi, minlength=cfg.nwin)
        assert cnt.max() <= cfg.EPW, f"bmax too small: {cnt.max()} > {cfg.EPW}"
        starts = np.zeros(cfg.nwin, np.int64)
        starts[1:] = np.cumsum(cnt)[:-1]
        pos = np.arange(len(dl)) - np.repeat(starts, cnt)
        slot = wi * cfg.EPW + pos
        srci_flat = np.zeros(cfg.EP, np.int32)
        srci_flat[slot] = src[em].astype(np.int32)
        dstf_flat = np.full(cfg.EP, -1.0, f32)
        dstf_flat[slot] = (dl - (wi << 7)).astype(f32)
        spE = np.zeros((cfg.EP, 256), np.float16)
        spE[slot] = sp[em]
        h_slice = h_pad[c * npc:(c + 1) * npc]
        m = dict(shared)
        m.update(
            hsT=np.ascontiguousarray(h_slice.T),
            h_sl=h_slice + W["bo"][None, :],
            spT=np.ascontiguousarray(spE.T),
            dstseq=dstf_flat[None, :],
            dstcol=np.ascontiguousarray(dstf_flat.reshape(-1, 128).T),
            srci=np.ascontiguousarray(srci_flat.reshape(-1, 128).T),
        )
        in_maps.append(m)
    return in_maps


def pick_bmax(cfg_nwin, npc, dst):
    core_of = dst // npc
    bmax = 1
    for c in range(NCORES):
        dl = dst[core_of == c] - c * npc
        if len(dl):
            cnt = np.bincount(dl >> 7, minlength=cfg_nwin)
            bmax = max(bmax, int(math.ceil(cnt.max() / 128)))
    return bmax


_CACHE = {}


def kernel(**inputs) -> np.ndarray:
    n_real, e_real = inputs["h"].shape[0], inputs["src"].shape[0]
    nwin = 49
    npc = nwin * 128
    dst = np.asarray(inputs["dst"]).astype(np.int64)
    bmax = pick_bmax(nwin, npc, dst)
    cfg = Cfg(nwin=nwin, bmax=bmax)
    in_maps = prepare(cfg, inputs, n_real, e_real)
    key = (cfg.nwin, cfg.bmax)
    if key not in _CACHE:
        _CACHE[key] = build(cfg)
    nc = _CACHE[key]
    res = run_bass_kernel_spmd(nc, in_maps, list(range(cfg.ncores)))
    out = np.concatenate([res.results[c]["out"] for c in range(cfg.ncores)], 0)
    return out[:n_real].astype(np.float32)


if __name__ == "__main__":
    pass



# revision 10
# speedup vs baseline: 1.0427x; 1.0427x over previous
"""Trainium2 Bass kernel for NodeGraphTransformerLayer (GNN message passing).

Strategy (8 NeuronCores, SPMD single program):
  - Pad node space to NPAD = 8 * NPC (NPC = nwin*128). Core c owns nodes
    [c*NPC, (c+1)*NPC) and ALL edges whose dst falls in that range, grouped by
    dst window. No cross-core reduction needed: each core computes its nodes'
    full output rows.
  - Host prep ("sharding"): partition edges per core, group per 128-node
    window, sort each window's edges by src (HBM locality for the gathers),
    pad each window's edge list to bmax 128-edge blocks, and precompute the
    per-edge spatial score sp8[e,h] = spatial_pos[e] @ Wsp_reduced + bsp_r
    (host BLAS; removes 55MB/core of spatial DMA and all spatial matmuls).
    Padding edges carry dst = -1 so they drop out of the one-hot segment sums
    (they gather row 0 harmlessly).
  - Device per core:
    Phase 1: KV table [NPAD, 512] f16 = h @ [Wk|Wv] + b (replicated on all
      cores, feeds gathers), written in 1024-row chunks; Q slice for own
      nodes, pre-scaled by 1/sqrt(HD), resident in SBUF.
    Phase 2 (per window w): one indirect-DMA gather per 128-edge block into a
      whole-window kvall tile; one-hot(dst) built once per window (f16), its
      transpose per block via DVE; per-edge score s = sum_h K*Q (DVE mult +
      reduce in 4-block batches through a 2-bank PSUM qe tile) + host sp8,
      clipped, exp'd on ScalarE once per window; messages [V*score | score]
      reduced into a PSUM accumulator via one-hot(dst) matmuls.
    Phase 3 (per window): h_attn = wV/(z+eps), sigmoid gate computed via the
      Exp table (1/(1+e^-x)) to avoid activation-table thrash, Wo, residual +
      LN1*BN1 (rstd via exp(-0.5*ln(var)) -- same Exp/Ln table), FFN (gelu),
      residual + LN2*BN2, DMA out. All phase-2/3 matmuls are f16.
"""

import math
import sys
from contextlib import ExitStack

import numpy as np

sys.path.insert(0, "/opt/trn_rl_repo")

import concourse.bass as bass
import concourse.tile as tile
from concourse import bacc, mybir
from concourse.bass import IndirectOffsetOnAxis
from concourse.bass_utils import run_bass_kernel_spmd

F32 = mybir.dt.float32
F16 = mybir.dt.float16
I32 = mybir.dt.int32
AF = mybir.ActivationFunctionType
ALU = mybir.AluOpType
AX = mybir.AxisListType

N, E, DIN, DOUT, H, HD, FF = 50000, 800000, 256, 256, 8, 32, 1024
NCORES = 8
SCALE = float(np.sqrt(DOUT // H))
EPS_LN = 1e-5
EPS_BN = 1e-5


class Cfg:
    def __init__(self, nwin, bmax, ncores=NCORES, npad=None):
        self.ncores = ncores
        self.nwin = nwin              # 128-node windows per core
        self.bmax = bmax              # 128-edge blocks per window (uniform)
        self.npc = nwin * 128         # padded nodes per core
        self.npad = npad if npad is not None else self.npc * ncores
        self.EPW = bmax * 128         # edge slots per window
        self.EP = nwin * self.EPW     # edge slots per core


def build(cfg: Cfg):
    nc = bacc.Bacc("TRN2", target_bir_lowering=False, debug=False,
                   num_devices=cfg.ncores)

    def inp(name, shape, dtype=F32):
        return nc.dram_tensor(name, list(shape), dtype, kind="ExternalInput")

    NB = cfg.nwin * cfg.bmax
    h_T = inp("h_T", [256, cfg.npad], F16)
    hsT_d = inp("hsT", [256, cfg.npc], F16)
    h_sl_d = inp("h_sl", [cfg.npc, 256], F16)    # h slice + bo, node-major
    sp8_d = inp("sp8", [128, NB * 8], F16)
    dstcol_d = inp("dstcol", [128, NB])
    srci_d = inp("srci", [128, NB], I32)
    Wkv = inp("Wkv", [256, 512], F16); bkv_row = inp("bkv_row", [1, 512], F16)
    Wq = inp("Wq", [256, 256], F16); bq_row = inp("bq_row", [1, 256], F16)
    Wg = inp("Wg", [512, 256], F16); nbgc = inp("nbgc", [128, 2])
    Wo = inp("Wo", [256, 256], F16)
    W1 = inp("W1", [256, 1024], F16); b1c = inp("b1c", [128, 8])
    W2 = inp("W2", [1024, 256], F16); b2t_d = inp("b2t", [128, 256], F16)
    cs1 = inp("cs1", [128, 256], F16); cb1 = inp("cb1", [128, 256], F16)
    cs2 = inp("cs2", [128, 256], F16); cb2 = inp("cb2", [128, 256], F16)
    iota_r = inp("iota_r", [128, 128]); ident = inp("ident", [128, 128], F16)
    ehead = inp("ehead", [8, 256], F16)
    ones_row = inp("ones_row", [1, 512], F16)
    out_d = nc.dram_tensor("out", [cfg.npc, 256], F32, kind="ExternalOutput")
    kvt = nc.dram_tensor("kv_table", [cfg.npad, 512], F16)

    with tile.TileContext(nc) as tc, ExitStack() as ctx, \
         nc.allow_low_precision(reason="f16 pipeline, rel tol 2e-2"):
        const = ctx.enter_context(tc.tile_pool(name="const", bufs=1))

        def ctile(src, shape, dtype=F32, tag=None, rearr=None):
            t = const.tile(list(shape), dtype, tag=tag or src.name)
            s = src[:]
            if rearr is not None:
                s = s.rearrange(rearr[0], **rearr[1])
            nc.sync.dma_start(t[:], s)
            return t

        kvw = ctile(Wkv, [128, 2, 512], dtype=F16, rearr=("(s p) n -> p s n", dict(p=128)))
        qw = ctile(Wq, [128, 2, 256], dtype=F16, rearr=("(s p) n -> p s n", dict(p=128)))
        wgw = ctile(Wg, [128, 4, 256], dtype=F16, rearr=("(s p) n -> p s n", dict(p=128)))
        wow = ctile(Wo, [128, 2, 256], dtype=F16, rearr=("(s p) n -> p s n", dict(p=128)))
        w1w = ctile(W1, [128, 2, 1024], dtype=F16, rearr=("(s p) n -> p s n", dict(p=128)))
        w2w = ctile(W2, [128, 8, 256], dtype=F16, rearr=("(s p) n -> p s n", dict(p=128)))
        bkvr = ctile(bkv_row, [1, 512], dtype=F16)
        bqr = ctile(bq_row, [1, 256], dtype=F16)
        nbgct = ctile(nbgc, [128, 2])
        b1ct = ctile(b1c, [128, 8])
        b2t = ctile(b2t_d, [128, 256], dtype=F16)
        cs1t = ctile(cs1, [128, 256], dtype=F16); cb1t = ctile(cb1, [128, 256], dtype=F16)
        cs2t = ctile(cs2, [128, 256], dtype=F16); cb2t = ctile(cb2, [128, 256], dtype=F16)
        iotar = ctile(iota_r, [128, 128])
        idt = ctile(ident, [128, 128], dtype=F16)
        eh = ctile(ehead, [8, 256], dtype=F16)
        onesr = ctile(ones_row, [1, 512], dtype=F16)
        srci_sb = ctile(srci_d, [128, NB], I32)
        dstc_sb = ctile(dstcol_d, [128, NB])
        sp8_sb = ctile(sp8_d, [128, NB * 8], dtype=F16)
        hsTr = ctile(hsT_d, [128, 2, cfg.npc], dtype=F16,
                     rearr=("(s p) n -> p s n", dict(p=128)))
        hslr = ctile(h_sl_d, [128, cfg.nwin, 256], dtype=F16,
                     rearr=("(w p) c -> p w c", dict(p=128)))
        qall = const.tile([128, cfg.nwin * 256], F16, tag="qall")
        zcol = const.tile([128, 1], F32, tag="zcol")
        nc.gpsimd.memset(zcol[:], 0.0)
        epscol = const.tile([128, 1], F32, tag="epscol")
        nc.gpsimd.memset(epscol[:], EPS_LN)
        nc.const_aps.aps[(F32, 0.0)] = zcol[:]
        nc.const_aps.aps[(F32, EPS_LN)] = epscol[:]

        # ---------------- phase 1: KV table + resident Q ----------------
        ST = 1024
        while cfg.npad % ST != 0:
            ST //= 2
        TPC = ST // 128
        with tc.tile_pool(name="p1", bufs=2) as p1, \
             tc.tile_pool(name="p1ps", bufs=2, space="PSUM") as p1ps, \
             tc.tile_pool(name="p1o", bufs=2) as p1o:
            for s in range(cfg.npad // ST):
                ht = p1.tile([128, 2, ST], F16, tag="ht")
                nc.sync.dma_start(ht[:, 0, :], h_T[0:128, s * ST:(s + 1) * ST])
                nc.sync.dma_start(ht[:, 1, :], h_T[128:256, s * ST:(s + 1) * ST])
                ot = p1o.tile([128, TPC, 512], F16, tag="kvo")
                for t in range(TPC):
                    ps = p1ps.tile([128, 512], F32, tag="kvps")
                    nc.tensor.matmul(ps[:], lhsT=ht[:, 0, t * 128:(t + 1) * 128],
                                     rhs=kvw[:, 0, :], start=True, stop=False)
                    nc.tensor.matmul(ps[:], lhsT=ht[:, 1, t * 128:(t + 1) * 128],
                                     rhs=kvw[:, 1, :], start=False, stop=False)
                    nc.tensor.matmul(ps[:], lhsT=onesr[0:1, 0:128],
                                     rhs=bkvr[0:1, :], start=False, stop=True)
                    nc.scalar.activation(out=ot[:, t, :], in_=ps[:], func=AF.Copy)
                nc.sync.dma_start(
                    kvt[s * ST:(s + 1) * ST, :].rearrange("(t p) c -> p t c", p=128),
                    ot[:])
            for w in range(cfg.nwin):
                ps = p1ps.tile([128, 256], F32, tag="qps")
                nc.tensor.matmul(ps[:], lhsT=hsTr[:, 0, w * 128:(w + 1) * 128],
                                 rhs=qw[:, 0, :], start=True, stop=False)
                nc.tensor.matmul(ps[:], lhsT=hsTr[:, 1, w * 128:(w + 1) * 128],
                                 rhs=qw[:, 1, :], start=False, stop=False)
                nc.tensor.matmul(ps[:], lhsT=onesr[0:1, 0:128],
                                 rhs=bqr[0:1, 0:256], start=False, stop=True)
                nc.scalar.activation(out=qall[:, w * 256:(w + 1) * 256],
                                     in_=ps[:], func=AF.Copy)

        # ---------------- phase 2+3 ----------------
        kvp = ctx.enter_context(tc.tile_pool(name="kvgp", bufs=2))
        p2 = ctx.enter_context(tc.tile_pool(name="p2", bufs=2))
        ps_wv = ctx.enter_context(tc.tile_pool(name="ps_wv", bufs=1, space="PSUM"))
        ps_qe = ctx.enter_context(tc.tile_pool(name="ps_qe", bufs=1, space="PSUM"))
        ps_b = ctx.enter_context(tc.tile_pool(name="ps_b", bufs=2, space="PSUM"))
        ps_g1 = ctx.enter_context(tc.tile_pool(name="ps_g1", bufs=1, space="PSUM"))
        p3 = ctx.enter_context(tc.tile_pool(name="p3", bufs=2))

        def layernorm(xin, cst, cbt, odt=F32):
            mu = p3.tile([128, 1], F32, tag="mu")
            nc.vector.tensor_reduce(out=mu[:], in_=xin[:], axis=AX.X, op=ALU.add)
            nc.vector.tensor_scalar_mul(out=mu[:], in0=mu[:], scalar1=1.0 / 256)
            xc = p3.tile([128, 256], F32, tag="xc")
            nc.vector.tensor_scalar(out=xc[:], in0=xin[:], scalar1=mu[:, 0:1],
                                    scalar2=None, op0=ALU.subtract)
            sq = p3.tile([128, 256], F32, tag="sq")
            nc.vector.tensor_tensor(out=sq[:], in0=xc[:], in1=xc[:], op=ALU.mult)
            vs = p3.tile([128, 1], F32, tag="vs")
            nc.vector.tensor_reduce(out=vs[:], in_=sq[:], axis=AX.X, op=ALU.add)
            lg = p3.tile([128, 1], F32, tag="lg")
            nc.scalar.activation(out=lg[:], in_=vs[:], func=AF.Ln,
                                 scale=1.0 / 256, bias=EPS_LN)
            rstd = p3.tile([128, 1], F32, tag="rstd")
            nc.scalar.activation(out=rstd[:], in_=lg[:], func=AF.Exp, scale=-0.5)
            xn = p3.tile([128, 256], F32, tag="xn")
            nc.vector.tensor_scalar(out=xn[:], in0=xc[:], scalar1=rstd[:, 0:1],
                                    scalar2=None, op0=ALU.mult)
            om = p3.tile([128, 256], F32, tag="lnm")
            nc.vector.tensor_tensor(out=om[:], in0=xn[:], in1=cst[:], op=ALU.mult)
            o = p3.tile([128, 256], odt, tag="lno")
            nc.vector.tensor_tensor(out=o[:], in0=om[:], in1=cbt[:], op=ALU.add)
            return o

        bm = cfg.bmax
        for w in range(cfg.nwin):
            c0 = w * bm
            kvall = kvp.tile([128, bm, 512], F16, tag="kvall")
            for b in range(bm):
                nc.gpsimd.indirect_dma_start(
                    out=kvall[:, b, :], out_offset=None, in_=kvt[:],
                    in_offset=IndirectOffsetOnAxis(
                        ap=srci_sb[:, c0 + b:c0 + b + 1], axis=0))
            oh = p2.tile([128, bm, 128], F16, tag="oh")
            nc.vector.tensor_tensor(
                out=oh[:],
                in0=dstc_sb[:, c0:c0 + bm].rearrange("p (b o) -> p b o", o=1)
                    .to_broadcast([128, bm, 128]),
                in1=iotar[:].rearrange("p (o n) -> p o n", o=1)
                    .to_broadcast([128, bm, 128]),
                op=ALU.is_equal)
            ohT = p2.tile([128, bm, 128], F16, tag="ohT")
            for b in range(bm):
                nc.vector.transpose(out=ohT[:, b, :], in_=oh[:, b, :])
            s_all = p2.tile([128, bm * 8], F32, tag="s_all")
            for g0 in range(0, bm, 4):
                gs = min(4, bm - g0)
                qe4 = ps_qe.tile([128, 4, 256], F32, tag="qe4")
                for j in range(gs):
                    nc.tensor.matmul(qe4[:, j, :], lhsT=ohT[:, g0 + j, :],
                                     rhs=qall[:, w * 256:(w + 1) * 256],
                                     start=True, stop=True, skip_group_check=True)
                tsb4 = p2.tile([128, 4, 256], F16, tag="tsb4")
                nc.vector.tensor_tensor(out=tsb4[:, 0:gs, :],
                                        in0=kvall[:, g0:g0 + gs, 0:256],
                                        in1=qe4[:, 0:gs, :], op=ALU.mult)
                nc.vector.tensor_reduce(
                    out=s_all[:, g0 * 8:(g0 + gs) * 8],
                    in_=tsb4[:, 0:gs, :].rearrange("p g (h d) -> p (g h) d", d=32),
                    axis=AX.X, op=ALU.add)
            nc.vector.tensor_tensor(out=s_all[:], in0=s_all[:],
                                    in1=sp8_sb[:, c0 * 8:(c0 + bm) * 8], op=ALU.add)
            nc.vector.tensor_scalar(out=s_all[:], in0=s_all[:],
                                    scalar1=5.0, scalar2=-5.0,
                                    op0=ALU.min, op1=ALU.max)
            scex = p2.tile([128, bm * 8], F16, tag="scex")
            nc.scalar.activation(out=scex[:], in_=s_all[:], func=AF.Exp)
            wv = ps_wv.tile([128, 384], F32, tag="wv")
            for g0 in range(0, bm, 4):
                gs = min(4, bm - g0)
                mext4 = p2.tile([128, 4, 256], F16, tag="mext4")
                nc.vector.tensor_tensor(
                    out=mext4[:, 0:gs, :].rearrange("p g (h d) -> p g h d", d=32),
                    in0=kvall[:, g0:g0 + gs, 256:512].rearrange(
                        "p g (h d) -> p g h d", d=32),
                    in1=scex[:, g0 * 8:(g0 + gs) * 8].rearrange(
                        "p (g h) -> p g h", h=8).to_broadcast([128, gs, 8, 32]),
                    op=ALU.mult)
                for j in range(gs):
                    b = g0 + j
                    st = b == 0
                    fin = b == bm - 1
                    nc.tensor.matmul(wv[:, 0:128], lhsT=mext4[:, j, 0:128],
                                     rhs=oh[:, b, :], start=st, stop=False,
                                     skip_group_check=True)
                    nc.tensor.matmul(wv[:, 128:256], lhsT=mext4[:, j, 128:256],
                                     rhs=oh[:, b, :], start=False, stop=False,
                                     skip_group_check=True)
                    nc.tensor.matmul(wv[0:8, 256:384], lhsT=scex[:, b * 8:(b + 1) * 8],
                                     rhs=oh[:, b, :], start=False, stop=fin,
                                     skip_group_check=True)

            # ---------------- phase 3 ----------------
            zr = p3.tile([8, 128], F32, tag="zr")
            nc.vector.tensor_scalar(out=zr[:], in0=wv[0:8, 256:384], scalar1=1e-6,
                                    scalar2=None, op0=ALU.add)
            zrr = p3.tile([8, 128], F16, tag="zrr")
            nc.vector.reciprocal(out=zrr[:], in_=zr[:])
            zrep = ps_b.tile([128, 256], F32, tag="psb")
            nc.tensor.matmul(zrep[:, 0:128], lhsT=eh[0:8, 0:128], rhs=zrr[:],
                             start=True, stop=False)
            nc.tensor.matmul(zrep[:, 128:256], lhsT=eh[0:8, 128:256], rhs=zrr[:],
                             start=False, stop=True)
            zrs = p3.tile([128, 256], F32, tag="zrs")
            nc.scalar.activation(out=zrs[:], in_=zrep[:], func=AF.Copy)
            hat = p3.tile([128, 256], F16, tag="hat")
            nc.vector.tensor_tensor(out=hat[:], in0=wv[:, 0:256], in1=zrs[:],
                                    op=ALU.mult)
            gate = ps_b.tile([128, 256], F32, tag="psb")
            rhs_list = [hsTr[:, 0, w * 128:(w + 1) * 128],
                        hsTr[:, 1, w * 128:(w + 1) * 128],
                        hat[:, 0:128], hat[:, 128:256]]
            for ci, rr in enumerate(rhs_list):
                for co in range(2):
                    nc.tensor.matmul(gate[:, co * 128:(co + 1) * 128],
                                     lhsT=wgw[:, ci, co * 128:(co + 1) * 128], rhs=rr,
                                     start=(ci == 0 and co == 0), stop=(ci == 3 and co == 1),
                                     skip_group_check=True)
            # sigmoid(g+bg) = 1/(1+exp(-g-bg)) -- stays on the Exp table
            eg = p3.tile([128, 256], F32, tag="eg")
            nc.scalar.activation(out=eg[:, 0:128], in_=gate[:, 0:128],
                                 func=AF.Exp, scale=-1.0, bias=nbgct[:, 0:1])
            nc.scalar.activation(out=eg[:, 128:256], in_=gate[:, 128:256],
                                 func=AF.Exp, scale=-1.0, bias=nbgct[:, 1:2])
            nc.vector.tensor_scalar(out=eg[:], in0=eg[:], scalar1=1.0,
                                    scalar2=None, op0=ALU.add)
            sg = p3.tile([128, 256], F32, tag="sg")
            nc.vector.reciprocal(out=sg[:], in_=eg[:])
            x1 = p3.tile([128, 256], F16, tag="x1")
            nc.vector.tensor_tensor(out=x1[:], in0=sg[:], in1=hat[:], op=ALU.mult)
            yps = ps_b.tile([128, 256], F32, tag="psb")
            nc.tensor.matmul(yps[:], lhsT=x1[:, 0:128], rhs=wow[:, 0, :],
                             start=True, stop=False)
            nc.tensor.matmul(yps[:], lhsT=x1[:, 128:256], rhs=wow[:, 1, :],
                             start=False, stop=True)
            x = p3.tile([128, 256], F32, tag="x")
            nc.vector.tensor_tensor(out=x[:], in0=yps[:], in1=hslr[:, w, :],
                                    op=ALU.add)
            x2in = layernorm(x, cs1t, cb1t, odt=F16)
            xT = ps_g1.tile([128, 256], F16, tag="psbT")
            nc.tensor.matmul(xT[:, 0:128], lhsT=x2in[:, 0:128], rhs=idt[:],
                             is_transpose=True, start=True, stop=False)
            nc.tensor.matmul(xT[:, 128:256], lhsT=x2in[:, 128:256], rhs=idt[:],
                             is_transpose=True, start=False, stop=True)
            xTs = p3.tile([128, 256], F16, tag="xTs")
            nc.scalar.activation(out=xTs[:], in_=xT[:], func=AF.Copy)
            g1s = p3.tile([128, 1024], F16, tag="g1s")
            for half in range(2):
                g1 = ps_g1.tile([128, 512], F32, tag="psg1")
                for q in range(4):
                    ct = half * 4 + q
                    off = q * 128
                    nc.tensor.matmul(g1[:, off:off + 128],
                                     lhsT=w1w[:, 0, ct * 128:(ct + 1) * 128],
                                     rhs=xTs[:, 0:128], start=(q == 0), stop=False,
                                     skip_group_check=True)
                    nc.tensor.matmul(g1[:, off:off + 128],
                                     lhsT=w1w[:, 1, ct * 128:(ct + 1) * 128],
                                     rhs=xTs[:, 128:256], start=False,
                                     stop=(q == 3), skip_group_check=True)
                for q in range(4):
                    ct = half * 4 + q
                    nc.scalar.activation(out=g1s[:, ct * 128:(ct + 1) * 128],
                                         in_=g1[:, q * 128:(q + 1) * 128],
                                         func=AF.Gelu, bias=b1ct[:, ct:ct + 1])
            x2p = ps_b.tile([128, 256], F32, tag="psb")
            for ct in range(8):
                nc.tensor.matmul(x2p[:], lhsT=g1s[:, ct * 128:(ct + 1) * 128],
                                 rhs=w2w[:, ct, :], start=(ct == 0), stop=(ct == 7))
            x3 = p3.tile([128, 256], F32, tag="x3")
            nc.vector.tensor_tensor(out=x3[:], in0=x2p[:], in1=x2in[:], op=ALU.add)
            nc.vector.tensor_tensor(out=x3[:], in0=x3[:], in1=b2t[:], op=ALU.add)
            xo = layernorm(x3, cs2t, cb2t)
            nc.sync.dma_start(out_d[w * 128:(w + 1) * 128, :], xo[:])

    nc.compile()
    return nc


def prepare(cfg: Cfg, inputs, n_real, e_real):
    """Host-side sharding: returns in_maps (list of dicts per core)."""
    f32 = np.float32
    f16 = np.float16
    h = np.asarray(inputs["h"], f32)
    sp = np.asarray(inputs["spatial_pos"], f32)
    src = np.asarray(inputs["src"]).astype(np.int64)
    dst = np.asarray(inputs["dst"]).astype(np.int64)
    W = {k: np.asarray(inputs[k], f32) for k in
         ["Wq", "bq", "Wk", "bk", "Wv", "bv", "Wsp", "bsp", "Wo", "bo",
          "Wg", "bg", "W1", "b1", "W2", "b2", "ln1_g", "ln1_b", "ln2_g",
          "ln2_b", "bn1_g", "bn1_b", "bn2_g", "bn2_b"]}

    npc, npad = cfg.npc, cfg.npad
    h_pad = np.zeros((npad, 256), f32)
    h_pad[:n_real] = h

    Wkv = np.concatenate([W["Wk"], W["Wv"]], 1)
    bkv = np.concatenate([W["bk"], W["bv"]])
    Wq_s = W["Wq"] / SCALE
    bq_s = W["bq"] / SCALE
    Wsp_r = W["Wsp"].astype(np.float64).reshape(256, 8, 32).sum(-1).astype(f32)
    bsp_r = W["bsp"].astype(np.float64).reshape(8, 32).sum(-1).astype(f32)
    # host spatial scores: [E, 8]
    sp8_full = (sp @ Wsp_r + bsp_r[None, :]).astype(f32)
    # reorder Wg rows: device concat layout [h(256) | h_attn(256)] ->
    # reference layout interleaved per head (h-head, attn-head)
    pr = np.empty(512, np.int64)
    r = np.arange(256)
    pr[:256] = (r // 32) * 64 + (r % 32)
    pr[256:] = (r // 32) * 64 + 32 + (r % 32)
    Wg_r = W["Wg"][pr]
    rs = 1.0 / np.sqrt(np.float32(1.0 + EPS_BN))
    cs1 = W["ln1_g"] * rs * W["bn1_g"]
    cb1 = W["ln1_b"] * rs * W["bn1_g"] + W["bn1_b"]
    cs2 = W["ln2_g"] * rs * W["bn2_g"]
    cb2 = W["ln2_b"] * rs * W["bn2_g"] + W["bn2_b"]

    rep = lambda v, dt=f16: np.tile(np.asarray(v, dt)[None, :], (128, 1))
    ehead = np.zeros((8, 256), f16)
    ehead[np.arange(256) // 32, np.arange(256)] = 1.0

    shared = dict(
        h_T=np.ascontiguousarray(h_pad.T).astype(f16),
        Wkv=Wkv.astype(f16), bkv_row=bkv[None, :].astype(f16),
        Wq=Wq_s.astype(f16), bq_row=bq_s[None, :].astype(f16),
        Wg=Wg_r.astype(f16),
        nbgc=np.ascontiguousarray((-W["bg"]).reshape(2, 128).T).astype(f32),
        Wo=W["Wo"].astype(f16),
        W1=W["W1"].astype(f16),
        b1c=np.ascontiguousarray(W["b1"].reshape(8, 128).T).astype(f32),
        W2=W["W2"].astype(f16), b2t=rep(W["b2"]),
        cs1=rep(cs1), cb1=rep(cb1), cs2=rep(cs2), cb2=rep(cb2),
        iota_r=np.tile(np.arange(128, dtype=f32), (128, 1)),
        ident=np.eye(128, dtype=f16),
        ehead=ehead,
        ones_row=np.ones((1, 512), f16),
    )

    core_of = dst // npc
    in_maps = []
    for c in range(cfg.ncores):
        em = np.nonzero(core_of == c)[0]
        dl = (dst[em] - c * npc).astype(np.int64)
        wi0 = dl >> 7
        # sort by (window, src) -- src order improves gather HBM locality
        order = np.lexsort((src[em], wi0))
        em = em[order]
        dl = dl[order]
        wi = dl >> 7
        cnt = np.bincount(wi, minlength=cfg.nwin)
        assert cnt.max() <= cfg.EPW, f"bmax too small: {cnt.max()} > {cfg.EPW}"
        starts = np.zeros(cfg.nwin, np.int64)
        starts[1:] = np.cumsum(cnt)[:-1]
        pos = np.arange(len(dl)) - np.repeat(starts, cnt)
        slot = wi * cfg.EPW + pos
        srci_flat = np.zeros(cfg.EP, np.int32)
        srci_flat[slot] = src[em].astype(np.int32)
        dstf_flat = np.full(cfg.EP, -1.0, f32)
        dstf_flat[slot] = (dl - (wi << 7)).astype(f32)
        sp8_flat = np.zeros((cfg.EP, 8), f16)
        sp8_flat[slot] = sp8_full[em].astype(f16)
        h_slice = h_pad[c * npc:(c + 1) * npc]
        m = dict(shared)
        m.update(
            hsT=np.ascontiguousarray(h_slice.T).astype(f16),
            h_sl=(h_slice + W["bo"][None, :]).astype(f16),
            sp8=np.ascontiguousarray(
                sp8_flat.reshape(-1, 128, 8).transpose(1, 0, 2).reshape(128, -1)),
            dstcol=np.ascontiguousarray(dstf_flat.reshape(-1, 128).T),
            srci=np.ascontiguousarray(srci_flat.reshape(-1, 128).T),
        )
        in_maps.append(m)
    return in_maps


def pick_bmax(cfg_nwin, npc, dst):
    core_of = dst // npc
    bmax = 1
    for c in range(NCORES):
        dl = dst[core_of == c] - c * npc
        if len(dl):
            cnt = np.bincount(dl >> 7, minlength=cfg_nwin)
            bmax = max(bmax, int(math.ceil(cnt.max() / 128)))
    return bmax


_CACHE = {}


def kernel(**inputs) -> np.ndarray:
    n_real, e_real = inputs["h"].shape[0], inputs["src"].shape[0]
    nwin = 49
    npc = nwin * 128
    dst = np.asarray(inputs["dst"]).astype(np.int64)
    bmax = pick_bmax(nwin, npc, dst)
    cfg = Cfg(nwin=nwin, bmax=bmax)
    in_maps = prepare(cfg, inputs, n_real, e_real)
    key = (cfg.nwin, cfg.bmax)
    if key not in _CACHE:
        _CACHE[key] = build(cfg)
    nc = _CACHE[key]
    res = run_bass_kernel_spmd(nc, in_maps, list(range(cfg.ncores)))
    out = np.concatenate([res.results[c]["out"] for c in range(cfg.ncores)], 0)
    return out[:n_real].astype(np.float32)


if __name__ == "__main__":
    pass
